# revision 23
# baseline (speedup 1.0000x reference)
"""ColBERTer forward as a Trainium2 Bass/Tile kernel, data-parallel over 8 cores.

Problem shapes (hardcoded): B=128, LQ=32, LD=512, H=768, C=128.

Strategy (v2: bf16 datapath)
----------------------------
Pure data parallel: batch dim sharded 16-per-core across 8 NeuronCores.
Host-side prep casts doc/query hidden states and W_comp to bf16 and re-lays
them out H-partitioned so the device needs ZERO on-chip transposes:

  docp[core][p, (b*6+ht)*512 + l] = doc_hidden[core*16+b, l, ht*128+p]   (bf16)
  qtp [core][p, ht*512 + b*32+q] = query_hidden[core*16+b, q, ht*128+p]  (bf16)

bf16 matmuls run at 1 PE cycle/row (fp32 needs 2 half-speed passes = 4x),
and the dominant doc DMA halves to ~12.6 MB/core, so the kernel sits at the
HBM roofline (target_regime=memory).

Per batch on device:
  d_tokT[c, l] = sum_ht W_tile[ht].T @ docT_tile[ht]   (6 accumulating matmuls)
  D = bf16(d_tokT + b_comp)                            (ACT copy w/ bias, bf16 out)
  psum_s[:, kt*33:+33] = D_kt.T @ [qv(b) | w_stop]     (4 matmuls: scores + imp col)
  imp[l]  = relu(imp_col + b_stop)                     (ACT, 4 cols)
  imm     = imp * dm                                   (DVE, 4 cols)
  sm      = psum_s * imm + (dm-1)*1000                 (DVE tensor_scalar dual op x4)
  term[q] = max over the 4 k-tiles                     (DVE max tree)

q_vecs for all 16 batches are computed once per core (bf16, bias folded in).
The qm masking of q_vecs is algebraically redundant (masked q rows are
dropped by the final where(qm,...) sum), so it is skipped on device.

Host-side epilogue: per-(b,q) max over the 128 partitions, cls score (exact
fp32 dot of CLS rows), qm-masked sum of term, sigmoid(score_merger) merge.
"""

import numpy as np
from contextlib import ExitStack

import concourse.bass as bass
import concourse.tile as tile
from concourse import mybir
from concourse import bass_utils

F32 = mybir.dt.float32
BF16 = mybir.dt.bfloat16
NP_BF16 = mybir.dt.np(BF16)
AF = mybir.ActivationFunctionType
ALU = mybir.AluOpType

B, LQ, LD, H, C = 128, 32, 512, 768, 128
NCORES = 8
BPC = B // NCORES       # 16 batches per core
HT = H // 128           # 6 h-tiles
KT = LD // 128          # 4 doc-token tiles
CHUNK = 2               # max batches per doc DMA chunk
CHUNK_SIZES = [2, 2, 2, 2, 2, 2, 2, 2]
EW = LQ + 1             # fused rhs width: 32 qv cols + 1 w_stop col

# cons16: bf16 consolidated W + q^T tensor (single DMA lane for all matmuls)
CONS_W = 0                       # [0, 768): W_comp as [hp, ht, c]
CONS_QT = HT * 128               # [768, 3840): q^T as [hp, ht, b*32+q]
CONS16_COLS = CONS_QT + HT * 512

# aux32: f32 small-constants tensor (single DMA lane for ACT/DVE operands)
A_BCOMP = 0
A_BSTOP = 1
A_WST16 = 2                      # w_stop replicated x16 (qvw col fill)
A_DMK = A_WST16 + BPC            # dm arranged [kp, b*4+kt]
A_OFFK = A_DMK + BPC * KT        # (dm-1)*1000
AUX_COLS = A_OFFK + BPC * KT

_CACHE = {}


# engine -> its own semaphore-name prefix (strict-FIFO compute queues only;
# a wait on the engine's OWN completion sem is an ordering no-op on these).
_OWN_SEM_PREFIX = {
    mybir.EngineType.PE: "PE_",
    mybir.EngineType.Activation: "Activation_",
    mybir.EngineType.DVE: "DVE_",
    mybir.EngineType.Pool: "Pool_",
}

# instruction types allowed to carry multiple waits (none on trn2 — every
# engine encoding holds a single sync-wait command)
_MULTIWAIT_OK = ()


def _fix_sync_waits(nc):
    """Enforce <=1 semaphore wait per engine instruction.

    The trn2 engine instruction encodings (S3_LW for matmul, S3D3_AC for
    activation, PSEUDO_DMA_DIRECT2D for HWDGE dma, ...) hold a single
    sync-wait command; walrus fails codegen with "Too many sync wait
    commands" otherwise. Two classes of redundant waits are dropped:

    1. own-engine waits: a wait on the instruction's own engine-completion
       semaphore. Compute queues execute and complete strictly in order
       (MATMULs are pc-monotone in start and end), so these are ordering
       no-ops emitted by Tile's bank-overlap guard.
    2. transitively-implied waits: wait (s2 >= v2) is dropped when another
       wait (s1 >= v1) of the same instruction implies it through the sem
       graph -- i.e. some instruction whose completion is counted in
       (s1 >= v1) itself waited on (s2 >= v2') with v2' >= v2 (closure
       computed over the whole program).

    Anything still >1 wait is a kernel-structure bug -- fail loudly at
    build time rather than at walrus codegen.
    """
    f = nc.m.functions[0]
    insts = [i for blk in f.blocks for i in blk.instructions]

    count = {}
    cover = {}
    stream_acc = {}

    def lookup(sem, val):
        """waits implied by 'sem has reached val'."""
        implied = {}
        for v_after, acc in cover.get(sem, []):
            if v_after <= val:
                implied.update(
                    {k: max(implied.get(k, -1), v) for k, v in acc.items()})
            else:
                break
        return implied

    # Per-compute-engine accumulated waits: engine queues execute and
    # complete strictly in order, so a sem update by instruction N implies
    # every wait carried by instructions 1..N of that engine — including
    # non-updating ones like InstLdweights (which carry the DMA wait for
    # the matmul that follows).
    eng_acc = {}

    for inst in insts:
        si = inst.sync_info
        waits = list(si.on_wait) if si is not None else []
        direct = {}
        for w in waits:
            direct[w.ant_name] = max(direct.get(w.ant_name, -1), w.wait_value)
            for k, v in lookup(w.ant_name, w.wait_value).items():
                direct[k] = max(direct.get(k, -1), v)
        if inst.engine in _OWN_SEM_PREFIX:
            acc_e = eng_acc.setdefault(inst.engine, {})
            for k, v in direct.items():
                acc_e[k] = max(acc_e.get(k, -1), v)
            direct = acc_e
        for u in (si.on_update if si is not None else []) or []:
            s = u.ant_name
            count[s] = count.get(s, 0) + u.update_value
            acc = stream_acc.setdefault(s, {})
            for k, v in direct.items():
                acc[k] = max(acc.get(k, -1), v)
            cover.setdefault(s, []).append((count[s], dict(acc)))

    for inst in insts:
        si = inst.sync_info
        if si is None or len(si.on_wait) <= 1:
            continue
        if isinstance(inst, _MULTIWAIT_OK):
            continue
        own = _OWN_SEM_PREFIX.get(inst.engine)
        kept = list(si.on_wait)
        if own is not None:
            kept = [w for w in kept if not w.ant_name.startswith(own)]
        if type(inst).__name__ == "InstDMACopy":
            # own-queue wait: a HWDGE queue processes its ring entries in
            # order, so a wait on the sem this DMA itself updates (its own
            # queue's completion sem) is a FIFO ordering no-op.
            own_q = {u.ant_name for u in (si.on_update or [])}
            kept = [w for w in kept if w.ant_name not in own_q]
        if len(kept) > 1:
            final = []
            for i, w in enumerate(kept):
                others = final + kept[i + 1:]
                if not any(
                    lookup(o.ant_name, o.wait_value).get(w.ant_name, -1) >= w.wait_value
                    for o in others
                ):
                    final.append(w)
            kept = final
        if len(kept) > 1:
            raise RuntimeError(
                f"{type(inst).__name__} {inst.name} still has {len(kept)} waits: "
                f"{[(w.ant_name, w.wait_value) for w in si.on_wait]}"
            )
        inst.sync_info = mybir.SyncInfo(on_wait=kept, on_update=si.on_update)


def _emit(nc: bass.Bass, fix_waits=True):
    docp = nc.dram_tensor("docp", [128, BPC * HT * 512], BF16, kind="ExternalInput").ap()
    cons16 = nc.dram_tensor("cons16", [128, CONS16_COLS], BF16, kind="ExternalInput").ap()
    aux32 = nc.dram_tensor("aux32", [128, AUX_COLS], F32, kind="ExternalInput").ap()
    # per-batch, per-k-tile column maxes; final max over the 128 partitions
    # happens on the host (avoids a PE transpose + partition reduction).
    mout = nc.dram_tensor("mout", [128, BPC * LQ], F32, kind="ExternalOutput").ap()

    with tile.TileContext(nc) as tc, ExitStack() as ctx:
        singles = ctx.enter_context(tc.tile_pool(name="singles", bufs=1))
        xp = ctx.enter_context(tc.tile_pool(name="xp", bufs=5))
        dp = ctx.enter_context(tc.tile_pool(name="dp", bufs=3))
        sp = ctx.enter_context(tc.tile_pool(name="sp", bufs=3))
        ssb = ctx.enter_context(tc.tile_pool(name="ssb", bufs=BPC))
        ip = ctx.enter_context(tc.tile_pool(name="ip", bufs=4))
        pw = ctx.enter_context(tc.tile_pool(name="pw", bufs=1, space="PSUM"))
        pq = ctx.enter_context(tc.tile_pool(name="pq", bufs=1, space="PSUM"))
        pd = ctx.enter_context(tc.tile_pool(name="pd", bufs=3, space="PSUM"))
        ps = ctx.enter_context(tc.tile_pool(name="ps", bufs=3, space="PSUM"))

        cons_sb = singles.tile([128, CONS16_COLS], BF16)
        aux_sb = singles.tile([128, AUX_COLS], F32)
        qvw_sb = singles.tile([128, BPC * EW], BF16)
        mo_sb = singles.tile([128, BPC * LQ], F32)
        touch_a = singles.tile([128, 1], F32)
        touch_v = singles.tile([128, 1], F32)

        # PE p-state warmup: the first ~9us of the kernel are preamble + DMA
        # issue with an idle PE, which leaves the tensor engine cold right
        # when the doc stream arrives. Run dummy matmuls on uninitialized
        # SBUF garbage (no DMA wait, results discarded) so PE enters the
        # stream at full speed. psum_qv reuses the bank afterwards (PE-own
        # ordering).
        N_WARM = 16
        garbage = singles.tile([128, LD], BF16)
        nc.vector.memset(garbage[:], 1.0)
        dummy = pw.tile([128, LD], F32, name="dummy")
        for i in range(N_WARM):
            nc.tensor.matmul(dummy[:], garbage[:, 0:128], garbage[:],
                             start=True, stop=True)

        nc.sync.dma_start(out=cons_sb[:], in_=cons16)
        nc.sync.dma_start(out=aux_sb[:], in_=aux32)
        # pre-observe the aux DMA lane on ACT and DVE so later consumers
        # don't need a second sync wait on their instruction.
        nc.scalar.copy(touch_a[:], aux_sb[:, 0:1])
        nc.vector.tensor_copy(touch_v[:], aux_sb[:, 0:1])

        w_sb = cons_sb[:, CONS_W:CONS_W + HT * 128]
        qt_sb = cons_sb[:, CONS_QT:CONS_QT + HT * 512]
        bcomp_ap = aux_sb[:, A_BCOMP:A_BCOMP + 1]
        bstop_ap = aux_sb[:, A_BSTOP:A_BSTOP + 1]

        # q_vecs^T (bf16, bias folded) for all 16 batches, interleaved with a
        # w_stop column per batch: qvw[:, b*33:(b+1)*33] = [q_vecs^T(b) | w_stop]
        qvw3 = qvw_sb[:].rearrange("p (b e) -> p b e", e=EW)
        psum_qv = pq.tile([128, BPC * LQ], F32)
        for ht in range(HT):
            nc.tensor.matmul(
                psum_qv[:],
                w_sb[:, ht * 128:(ht + 1) * 128],
                qt_sb[:, ht * 512:(ht + 1) * 512],
                start=(ht == 0),
                stop=(ht == HT - 1),
            )
        nc.scalar.activation(
            qvw3[:, :, 0:LQ],
            psum_qv[:].rearrange("p (b q) -> p b q", q=LQ),
            AF.Identity, bias=bcomp_ap, scale=1.0,
        )
        nc.scalar.copy(qvw3[:, :, LQ:EW],
                       aux_sb[:, A_WST16:A_WST16 + BPC].rearrange("p (b o) -> p b o", o=1))



        # doc chunk DMAs are emitted one chunk AHEAD of their compute (and
        # the first two before any compute) so the per-chunk mout DMA never
        # head-of-line-blocks the next doc chunk on the queue.
        nchunks = len(CHUNK_SIZES)
        starts = np.cumsum([0] + CHUNK_SIZES).tolist()
        xts = [None] * nchunks

        def dma_chunk(k):
            nb = CHUNK_SIZES[k]
            xts[k] = xp.tile([128, CHUNK * HT * 512], BF16, tag="xt", name="xt")
            lo = starts[k] * HT * 512
            nc.sync.dma_start(out=xts[k][:, 0:nb * HT * 512],
                              in_=docp[:, lo:lo + nb * HT * 512])

        dma_chunk(0)
        dma_chunk(1)

        dma_chunk(2)
        for k in range(nchunks):
            if k + 3 < nchunks:
                dma_chunk(k + 3)
            xt = xts[k]
            if k >= 1:
                # p-state filler: absorb the chunk-boundary DMA wait with
                # dummy matmuls so the tensor engine never idles (an idle
                # gap resets PE to the half-speed pipeline state for ~3us).
                for _ in range(2):
                    nc.tensor.matmul(dummy[:], garbage[:, 0:128], garbage[:],
                                     start=True, stop=True)

            # compressor for the whole chunk, ht-outer: one W[ht] stationary
            # load serves every batch in the chunk (amortizes Ldweights +
            # PE SBUF access latency across back-to-back matmuls)
            nb = CHUNK_SIZES[k]
            psum_ds = []
            for bi in range(nb):
                psum_d = pd.tile([128, LD], F32, tag="pd", name="psum_d")
                psum_ds.append(psum_d)
            for ht in range(HT):
                for bi in range(nb):
                    o = (bi * HT + ht) * 512
                    nc.tensor.matmul(
                        psum_ds[bi][:],
                        w_sb[:, ht * 128:(ht + 1) * 128],
                        xt[:, o:o + 512],
                        start=(ht == 0),
                        stop=(ht == HT - 1),
                    )

            for bi in range(CHUNK_SIZES[k]):
                gb = starts[k] + bi
                d_sb = dp.tile([128, LD], BF16)
                nc.scalar.activation(d_sb[:], psum_ds[bi][:], AF.Identity,
                                     bias=bcomp_ap, scale=1.0)

                # fused raw-scores^T + importance column, per k-tile:
                # psum_s[:, kt*33:(kt+1)*33] = d_sb[:, kt].T @ [qv(b) | w_stop]
                psum_s = ps.tile([128, KT * EW], F32)
                for kt in range(KT):
                    nc.tensor.matmul(
                        psum_s[:, kt * EW:(kt + 1) * EW],
                        d_sb[:, kt * 128:(kt + 1) * 128],
                        qvw_sb[:, gb * EW:(gb + 1) * EW],
                        start=True,
                        stop=True,
                    )

                # single PSUM->SBUF copy on ACT (only ACT may read PSUM: DVE
                # psum reads race the matmul writes); everything downstream
                # runs on DVE from SBUF.
                s_sb = ssb.tile([128, KT * EW], F32)
                nc.scalar.copy(s_sb[:], psum_s[:])
                s3 = s_sb[:].rearrange("p (kt e) -> p kt e", e=EW)

                # importance = relu(imp_col + b_stop), per-partition (=doc pos)
                imp_sb = ip.tile([128, KT], F32, tag="imp")
                nc.vector.tensor_scalar(
                    imp_sb[:].rearrange("p (kt o) -> p kt o", o=1),
                    s3[:, :, LQ:EW],
                    bstop_ap, 0.0, ALU.add, ALU.max,
                )

                # masking, fused + exact via a +1000 shift (host subtracts):
                #   u  = s * imp                      (imp broadcast over q)
                #   sm = (u + 1000) * dm              (dm broadcast over q)
                # masked (dm=0) positions become 0 = (-1000 + 1000): exactly
                # the reference's where(dm, s, -1000) after the host's -1000.
                imprep = imp_sb[:].unsqueeze(2).broadcast_to((128, KT, LQ))
                dmrep = aux_sb[:, A_DMK + gb * KT:A_DMK + (gb + 1) * KT] \
                    .unsqueeze(2).broadcast_to((128, KT, LQ))
                sm_t = sp.tile([128, KT * LQ], F32, tag="sm", name="sm")
                sm3 = sm_t[:].rearrange("p (kt q) -> p kt q", q=LQ)
                nc.vector.tensor_mul(sm3[:], s3[:, :, 0:LQ], imprep)
                nc.vector.scalar_tensor_tensor(
                    sm3[:], sm3[:], 1000.0, dmrep, ALU.add, ALU.mult)
                sm = sm3.rearrange("p kt q -> p (kt q)")

                # max over the 4 k-tiles (DVE); host finishes max over partitions
                m1 = sp.tile([128, 2 * LQ], F32, tag="m1")
                nc.vector.tensor_max(m1[:], sm[:, 0:2 * LQ], sm[:, 2 * LQ:4 * LQ])
                nc.vector.tensor_max(mo_sb[:, gb * LQ:(gb + 1) * LQ],
                                     m1[:, 0:LQ], m1[:, LQ:2 * LQ])

        nc.sync.dma_start(out=mout, in_=mo_sb[:])
    if fix_waits:
        _fix_sync_waits(nc)
    return nc


def _get_nc(fix_waits=True):
    key = ("nc", fix_waits)
    if key not in _CACHE:
        nc = bass.Bass("TRN2", target_bir_lowering=False, debug=False,
                       num_devices=NCORES)
        _emit(nc, fix_waits=fix_waits)
        _CACHE[key] = nc
    return _CACHE[key]


def make_in_maps(query_hidden, doc_hidden, query_mask, doc_mask,
                 W_comp, b_comp, w_stop, b_stop, score_merger):
    """Host-side shard + relayout + bf16 cast. Returns list of 8 in_maps."""
    q = np.asarray(query_hidden, dtype=np.float32).astype(NP_BF16)
    d = np.asarray(doc_hidden, dtype=np.float32).astype(NP_BF16)
    W = np.asarray(W_comp, dtype=np.float32).astype(NP_BF16)

    # doc: (core, b, l, ht, hp) -> (core, hp, b, ht, l)
    docp = np.ascontiguousarray(
        d.reshape(NCORES, BPC, LD, HT, 128).transpose(0, 4, 1, 3, 2)
    ).reshape(NCORES, 128, BPC * HT * 512)

    # query: (core, b, q, ht, hp) -> (core, hp, ht, b, q)
    qtp = np.ascontiguousarray(
        q.reshape(NCORES, BPC, LQ, HT, 128).transpose(0, 4, 3, 1, 2)
    ).reshape(NCORES, 128, HT * 512)

    # W: (ht, hp, c) -> (hp, ht, c)
    wp = np.ascontiguousarray(W.reshape(HT, 128, C).transpose(1, 0, 2)).reshape(128, HT * 128)

    cons = np.zeros((NCORES, 128, CONS16_COLS), dtype=NP_BF16)
    cons[:, :, CONS_W:CONS_W + HT * 128] = wp[None]
    cons[:, :, CONS_QT:CONS_QT + HT * 512] = qtp

    dm_f = np.asarray(doc_mask).astype(np.float32)
    # (core, b, kt, kp) -> (core, kp, b, kt)
    dmk = np.ascontiguousarray(
        dm_f.reshape(NCORES, BPC, KT, 128).transpose(0, 3, 1, 2)
    ).reshape(NCORES, 128, BPC * KT)
    offk = (dmk - 1.0) * 1000.0

    aux = np.zeros((NCORES, 128, AUX_COLS), dtype=np.float32)
    aux[:, :, A_BCOMP] = np.asarray(b_comp, dtype=np.float32)[None, :]
    aux[:, :, A_BSTOP] = np.float32(np.asarray(b_stop, dtype=np.float32)[0])
    aux[:, :, A_WST16:A_WST16 + BPC] = np.asarray(w_stop, dtype=np.float32)[None, :, 0:1]
    aux[:, :, A_DMK:A_DMK + BPC * KT] = dmk
    aux[:, :, A_OFFK:A_OFFK + BPC * KT] = offk

    in_maps = []
    for c in range(NCORES):
        in_maps.append({
            "docp": np.ascontiguousarray(docp[c]),
            "cons16": np.ascontiguousarray(cons[c]),
            "aux32": np.ascontiguousarray(aux[c]),
        })
    return in_maps


def host_epilogue(mout_list, query_hidden, doc_hidden, query_mask, score_merger):
    """mout_list: list of 8 [128, BPC*LQ] arrays (per-k-tile column maxes)."""
    term = np.concatenate(
        [m.reshape(128, BPC, LQ).max(axis=0) for m in mout_list], axis=0
    ) - np.float32(1000.0)  # [B, LQ]; device values are shifted by +1000
    qm = np.asarray(query_mask).astype(bool)
    term_score = np.where(qm, term, np.float32(0.0)).astype(np.float32).sum(axis=-1, dtype=np.float32)

    q_cls = np.asarray(query_hidden, dtype=np.float32)[:, 0, :]
    d_cls = np.asarray(doc_hidden, dtype=np.float32)[:, 0, :]
    cls_score = np.sum(q_cls * d_cls, axis=-1, dtype=np.float32)

    sm = np.float32(np.asarray(score_merger, dtype=np.float32)[0])
    w = np.float32(1.0) / (np.float32(1.0) + np.exp(-sm, dtype=np.float32))
    cls_out = (cls_score * w).astype(np.float32)
    term_out = (term_score * (np.float32(1.0) - w)).astype(np.float32)
    score = (cls_out + term_out).astype(np.float32)
    return score, cls_out, term_out


def kernel(query_hidden, doc_hidden, query_mask, doc_mask,
           W_comp, b_comp, w_stop, b_stop, score_merger):
    nc = _get_nc()
    in_maps = make_in_maps(query_hidden, doc_hidden, query_mask, doc_mask,
                           W_comp, b_comp, w_stop, b_stop, score_merger)
    res = bass_utils.run_bass_kernel_spmd(nc, in_maps, core_ids=list(range(NCORES)))
    mout_list = [res.results[c]["mout"] for c in range(NCORES)]
    return host_epilogue(mout_list, query_hidden, doc_hidden, query_mask, score_merger)
